# revision 1
# baseline (speedup 1.0000x reference)
"""AttentionBlock (GroupNorm -> qkv 1x1 -> 8-head attention over 64x64 px -> proj
-> residual) on 8 Trainium2 NeuronCores, written in Bass/Tile.

Sharding: head-parallel. Core h computes head h end-to-end (each core loads the
full x), then one AllToAll reshards the attention output from head-parallel to
pixel-parallel and each core computes the output projection + residual for its
own 512-pixel slice (output concatenated on host).

Key techniques:
- GroupNorm is folded into the qkv weights on-device: per-channel scale
  s_c = rsqrt(var_g + eps) is multiplied into W (per-input-channel), and the
  mean term becomes a per-output bias b_eff = b - W''mu. x is never
  normalized explicitly; rsqrt is computed as exp(-0.5*ln(v+eps)) so the
  whole kernel uses a single ACT table set.
- Attention computes S^T = K^T.T @ Q^T with keys on PSUM partitions and
  queries on the free axis, so softmax needs no max-subtraction (|S| <= ~6)
  and no transposes: exp() goes PSUM->SBUF on the Scalar engine in
  [128, 1536] tiles, and P^T feeds the PV matmul directly.
- The softmax denominator comes for free as a 65th "ones" column in the
  V stationary operand of the PV matmul; O^T rows are rescaled by the
  reciprocal broadcast via a K=1 matmul (avoids cross-partition ops).
- S^T matmuls are row-paired with tile_position (64,0) via duplicated
  Q^T/K^T partitions 64-127 (host-duplicated weight columns), running two
  K=64 matmuls concurrently on the PE array.
- Compute dtypes: S^T/QK in float32r (full-rate fp32, ~1e-4), x/V/P in
  bf16; exp inputs fp32 from PSUM. Final rel err ~3e-4.
- Emission is software-pipelined so k-chunk/V-batch producers land just
  before their attention consumers and exp runs gap-free across q-blocks.
"""

import warnings

warnings.filterwarnings("ignore")

import numpy as np

N_CORES = 8
C = 512
HW = 4096
HD = 64
PXS = HW // N_CORES  # 512 pixels per core for the proj phase
EPS = 1e-6
GROUPS = [2] + [3] * 10  # k-tile group sizes per exp op (32 k-tiles; small group first)

_CACHE = {}


def build(with_collective=True):
    import concourse.bass as bass
    import concourse.bacc as bacc
    import concourse.mybir as mybir
    import concourse.tile as tile

    f32 = mybir.dt.float32
    f32r = mybir.dt.float32r
    bf16 = mybir.dt.bfloat16
    AF = mybir.ActivationFunctionType
    OP = mybir.AluOpType

    nc = bacc.Bacc("TRN2", target_bir_lowering=False, debug=False,
                   num_devices=N_CORES)

    persist_holder = {}

    def T(shape, dtype, name):
        return persist_holder["pool"].tile(shape, dtype, tag=name, name=name)

    # ---- DRAM I/O ----
    x_r = nc.dram_tensor("x_r", [C, HW], bf16, kind="ExternalInput")
    xs_d = nc.dram_tensor("xs", [C, PXS], f32, kind="ExternalInput")
    wq2_d = nc.dram_tensor("wq2", [4, 128, 128], bf16, kind="ExternalInput")
    wk2_d = nc.dram_tensor("wk2", [4, 128, 128], bf16, kind="ExternalInput")
    wv_d = nc.dram_tensor("wv", [4, 128, 64], bf16, kind="ExternalInput")
    bq2_d = nc.dram_tensor("bq2", [128, 1], f32, kind="ExternalInput")
    bk2_d = nc.dram_tensor("bk2", [128, 1], f32, kind="ExternalInput")
    bvr_d = nc.dram_tensor("bvr", [1, 65], f32, kind="ExternalInput")
    g4_d = nc.dram_tensor("g4", [4, 128, 32], f32, kind="ExternalInput")
    b4_d = nc.dram_tensor("b4", [4, 32, 128], f32, kind="ExternalInput")
    onesc_d = nc.dram_tensor("onesc", [128, 32], bf16, kind="ExternalInput")
    o64_d = nc.dram_tensor("o64", [1, 64], f32r, kind="ExternalInput")
    onesr_d = nc.dram_tensor("onesr", [1, 128], f32r, kind="ExternalInput")
    pw_d = nc.dram_tensor("pw", [4, 4, 128, 128], bf16, kind="ExternalInput")
    pb_d = nc.dram_tensor("pb", [128, 4], f32, kind="ExternalInput")
    out_d = nc.dram_tensor("out", [C, PXS], f32, kind="ExternalOutput")

    with tile.TileContext(nc) as tc:
      with tc.tile_pool(name="persist", bufs=1) as persist:
        persist_holder["pool"] = persist
        # ---------- persistent SBUF ----------
        xt = [T([128, HW], bf16, name=f"xt{t}") for t in range(4)]
        q2 = T([128, HW], f32r, name="q2")
        k2 = T([128, HW], f32r, name="k2")
        v_sb = T([128, 32 * 65], bf16, name="v_sb")
        otbig = T([128, HW], bf16, name="otbig")
        ot = otbig[0:64, :]
        wq2 = [T([128, 128], bf16, name=f"wq2_{t}") for t in range(4)]
        wk2 = [T([128, 128], bf16, name=f"wk2_{t}") for t in range(4)]
        wv = [T([128, 64], bf16, name=f"wv_{t}") for t in range(4)]
        pw = [[T([128, 128], bf16, name=f"pw_{ci}_{oi}") for oi in range(4)]
              for ci in range(4)]
        ones32 = T([128, 32], bf16, name="ones32")
        o64 = T([128, 64], f32r, name="o64")
        onesr_big = T([128, 128], f32r, name="onesr_big")
        bvb_big = T([128, 512], f32r, name="bvb_big")
        bq2 = T([128, 1], f32, name="bq2")
        bk2 = T([128, 1], f32, name="bk2")
        row_fb = T([128, 130], f32, name="row_fb")
        row_f = row_fb[0:1, :]  # 0:65 bvr, 65:130 bv_eff
        g4 = [T([128, 32], f32, name=f"g4_{t}") for t in range(4)]
        b4big = T([128, 512], f32, name="b4big")
        b4all = b4big[0:32, :]
        pb = T([128, 4], f32, name="pb")
        xs = [T([128, PXS], f32, name=f"xs{t}") for t in range(4)]

        # ---------- loads (x first, in quarters so stats can chase the DMA) ----------
        for t in range(4):
            for q in range(4):
                nc.sync.dma_start(xt[t][:, 1024 * q:1024 * (q + 1)],
                                  x_r.ap()[128 * t:128 * (t + 1),
                                           1024 * q:1024 * (q + 1)])
        for t in range(4):
            nc.sync.dma_start(g4[t][:], g4_d.ap()[t])
            nc.sync.dma_start(b4all[:, 128 * t:128 * (t + 1)], b4_d.ap()[t])
            nc.sync.dma_start(wq2[t][:], wq2_d.ap()[t])
            nc.sync.dma_start(wk2[t][:], wk2_d.ap()[t])
            nc.sync.dma_start(wv[t][:], wv_d.ap()[t])
        nc.sync.dma_start(bq2[:], bq2_d.ap())
        nc.sync.dma_start(bk2[:], bk2_d.ap())
        nc.sync.dma_start(row_f[:, 0:65], bvr_d.ap())
        nc.sync.dma_start(ones32[:], onesc_d.ap())
        nc.sync.dma_start(o64[64:65, :], o64_d.ap())
        nc.sync.dma_start(onesr_big[0:1, :], onesr_d.ap())

        # ---------- phase A: group-norm statistics ----------
        bq_eff = T([128, 1], f32, name="bq_eff")
        bk_eff = T([128, 1], f32, name="bk_eff")

        st_s = [T([128, 1], f32, name=f"st_s{t}") for t in range(4)]
        st_t = [T([128, 1], bf16, name=f"st_t{t}") for t in range(4)]

        with tc.tile_pool(name="psSa", bufs=1, space="PSUM") as psSa, \
             tc.tile_pool(name="psSb", bufs=1, space="PSUM") as psSb, \
             tc.tile_pool(name="psO", bufs=1, space="PSUM") as psO, \
             tc.tile_pool(name="ps1", bufs=1, space="PSUM") as ps1, \
             tc.tile_pool(name="ptp", bufs=8) as ptp, \
             tc.tile_pool(name="dram", bufs=1, space="DRAM") as dram:
            # tiles 0-1: raw sum/sumsq on the otherwise-idle ACT engine
            # (g4 carries the extra 1/4096 for these); tiles 2-3: bn_stats on DVE
            sqs = T([128, HW], bf16, name="sqs")
            e2 = [T([128, 2], f32, name=f"e2{t}") for t in range(4)]
            # dummy Square on a preamble constant: hoists the first ACT table
            # load ahead of the x-DMA wait (same func as the first real op, so
            # the table-set choice is unchanged)
            one_c = nc.const_aps.scalar_like(1.0, sqs[0:1, 0:1])
            nc.scalar.activation(sqs[0:1, 0:1], one_c, AF.Square)
            for t in range(2):
                nc.scalar.activation(sqs[:], xt[t][:], AF.Square,
                                     accum_out=e2[t][:, 1:2])
                nc.scalar.activation(sqs[:], xt[t][:], AF.Copy,
                                     accum_out=e2[t][:, 0:1])
            # dummy Ln after the stats ops: its natural_log table load runs
            # during the stats combine instead of inside the rsqrt chain
            nc.scalar.activation(sqs[0:1, 1:2], one_c, AF.Ln)
            bno = [T([128, 48], f32, name=f"bno{t}") for t in range(2, 4)]
            mv = [T([128, 2], f32, name=f"mv{t}") for t in range(2, 4)]
            for i, t in enumerate([2, 3]):
                xf = xt[t][:]
                for j in range(8):
                    nc.vector.bn_stats(bno[i][:, 6 * j:6 * j + 6],
                                       xf[:, 512 * j:512 * (j + 1)])
                nc.vector.bn_aggr(mv[i][:], bno[i][:].rearrange("p (a b) -> p a b", b=6))
                nc.vector.tensor_copy(e2[t][:, 0:1], mv[i][:, 0:1])
                nc.vector.tensor_tensor(e2[t][:, 1:2], mv[i][:, 0:1], mv[i][:, 0:1],
                                        op=OP.mult)
                nc.vector.tensor_tensor(e2[t][:, 1:2], e2[t][:, 1:2], mv[i][:, 1:2],
                                        op=OP.add)
            ps_st = psSa.tile([32, 2], f32, tag="s", name="ps_st")
            for t in range(4):
                nc.tensor.matmul(ps_st[:], g4[t][:], e2[t][:],
                                 start=(t == 0), stop=(t == 3))
            # sg cols: 0 = mean_g, 1 = E[x^2]_g, 2 = var_g, 3 = ln(var+eps)
            sgbig = T([128, 6], f32, name="sgbig")
            sgall = sgbig[0:32, :]
            sg = sgall
            nc.vector.tensor_copy(sg[:, 0:2], ps_st[:])
            nc.vector.tensor_tensor(sg[:, 2:3], sg[:, 0:1], sg[:, 0:1], op=OP.mult)
            nc.vector.tensor_tensor(sg[:, 2:3], sg[:, 1:2], sg[:, 2:3], op=OP.subtract)
            nc.vector.tensor_scalar_add(sg[:, 2:3], sg[:, 2:3], EPS)
            nc.scalar.activation(sg[:, 3:4], sg[:, 2:3], AF.Ln)
            nc.scalar.activation(sg[:, 4:5], sg[:, 3:4], AF.Exp, scale=-0.5)
            nc.vector.tensor_copy(sg[:, 5:6], sg[:, 0:1])
            # broadcast group -> channel
            for t in range(4):
                ps_bc = (psSb if t % 2 else psSa).tile([128, 2], f32, tag="s", name=f"ps_bc{t}")
                nc.tensor.matmul(ps_bc[:], b4all[:, 128 * t:128 * (t + 1)], sgall[:, 4:6], start=True, stop=True)
                nc.vector.tensor_copy(st_s[t][:], ps_bc[:, 0:1])
                nc.vector.tensor_copy(st_t[t][:], ps_bc[:, 1:2])
            # scale weights in place first: W'' = W' * s_c (qk first, v later)
            for t in range(4):
                nc.vector.tensor_scalar_mul(wk2[t][:], wk2[t][:], st_s[t][:])
                nc.vector.tensor_scalar_mul(wq2[t][:], wq2[t][:], st_s[t][:])
            for t in range(4):
                nc.vector.tensor_scalar_mul(wv[t][:], wv[t][:], st_s[t][:])
            # effective biases: b_eff = b' - W'' @ mu
            ps_bq = psSb.tile([128, 1], f32, tag="s", name="ps_bq")
            for t in range(4):
                nc.tensor.matmul(ps_bq[:], wq2[t][:], st_t[t][:],
                                 start=(t == 0), stop=(t == 3))
            nc.vector.scalar_tensor_tensor(bq_eff[:], ps_bq[:], -1.0, bq2[:],
                                           op0=OP.mult, op1=OP.add)
            ps_bk = psSa.tile([128, 1], f32, tag="s", name="ps_bk")
            for t in range(4):
                nc.tensor.matmul(ps_bk[:], wk2[t][:], st_t[t][:],
                                 start=(t == 0), stop=(t == 3))
            nc.vector.scalar_tensor_tensor(bk_eff[:], ps_bk[:], -1.0, bk2[:],
                                           op0=OP.mult, op1=OP.add)
            ps_bv = psSb.tile([1, 64], f32, tag="s", name="ps_bv")
            for t in range(4):
                nc.tensor.matmul(ps_bv[:], st_t[t][:], wv[t][:],
                                 start=(t == 0), stop=(t == 3))
            nc.vector.scalar_tensor_tensor(row_f[:, 65:129], ps_bv[:], -1.0,
                                           row_f[:, 0:64], op0=OP.mult, op1=OP.add)
            for j8 in range(8):
                nc.vector.tensor_copy(bvb_big[0:1, 64 * j8:64 * (j8 + 1)],
                                      row_f[:, 65:129])

            # ---------- phase B: qkv (k first; V batched 8 px-tiles per bank) ----------
            v_ones_view = v_sb[:].rearrange("p (t e) -> p t e", e=65)[:, :, 64]
            nc.vector.tensor_copy(v_ones_view, ones32[:])

            def emit_k(p):
                sl = slice(512 * p, 512 * (p + 1))
                pk = ps1.tile([128, 512], f32, tag="t", name=f"pk{p}")
                for t in range(4):
                    nc.tensor.matmul(pk[:], wk2[t][:], xt[t][:, sl],
                                     start=(t == 0), stop=(t == 3))
                nc.vector.tensor_scalar_add(k2[:, sl], pk[:], bk_eff[:])

            def emit_q(p):
                sl = slice(512 * p, 512 * (p + 1))
                pq = ps1.tile([128, 512], f32, tag="t", name=f"pq{p}")
                for t in range(4):
                    nc.tensor.matmul(pq[:], wq2[t][:], xt[t][:, sl],
                                     start=(t == 0), stop=(t == 3))
                nc.vector.tensor_scalar_add(q2[:, sl], pq[:], bq_eff[:])

            def emit_vbatch(b):
                pvb = psO.tile([128, 512], f32, tag="po", name=f"pvb{b}")
                nc.tensor.matmul(pvb[:], onesr_big[0:1, :], bvb_big[0:1, :],
                                 start=True, stop=False)
                for s in range(8):
                    pt_i = 8 * b + s
                    for t in range(4):
                        nc.tensor.matmul(pvb[:, 64 * s:64 * (s + 1)],
                                         xt[t][:, 128 * pt_i:128 * (pt_i + 1)],
                                         wv[t][:], start=False,
                                         stop=(s == 7 and t == 3))
                vv = v_sb[:].rearrange("p (n e) -> p n e", e=65)
                nc.vector.tensor_copy(
                    vv[:, 8 * b:8 * (b + 1), 0:64],
                    pvb[:].rearrange("p (n e) -> p n e", e=64))


            # ---------- phase C: attention ----------
            a2a_in = dram.tile([N_CORES, 64, PXS], bf16, name="a2a_in")
            a2a_out = dram.tile([N_CORES, 64, PXS], bf16, name="a2a_out")
            rball = T([128, 512], f32r, name="rball")
            rsb = T([128, 1024], f32, name="rsb")

            GSTART = []
            acc = 0
            for gs in GROUPS:
                GSTART.append(acc)
                acc += gs
            NG = len(GROUPS)
            pt_tiles = {}

            def emit_st_exp(qb, gi):
                gs = GROUPS[gi]
                k0 = GSTART[gi]
                qsl = slice(512 * qb, 512 * (qb + 1))
                pool = psSa if (qb * NG + gi) % 2 == 0 else psSb
                ps_s = pool.tile([128, 512 * gs], f32, tag="s", name=f"ps_s_{qb}_{gi}")
                pt_t = ptp.tile([128, 1536], bf16, tag="p", name=f"pt_{qb}_{gi}")
                pt_tiles[(qb, gi)] = pt_t
                for j in range(gs):
                    kt = k0 + j
                    hb = 64 * (kt % 2)
                    nc.tensor.matmul(
                        ps_s[:, 512 * j:512 * (j + 1)],
                        k2[hb:hb + 64, 128 * kt:128 * (kt + 1)],
                        q2[hb:hb + 64, qsl], start=True, stop=True)
                nc.scalar.activation(pt_t[:, :512 * gs], ps_s[:, :512 * gs], AF.Exp)

            def emit_pv(qb, gi, po):
                gs = GROUPS[gi]
                k0 = GSTART[gi]
                pt_t = pt_tiles.pop((qb, gi))
                for j in range(gs):
                    ki = k0 + j
                    nc.tensor.matmul(po[:], v_sb[:, 65 * ki:65 * (ki + 1)],
                                     pt_t[:, 512 * j:512 * (j + 1)],
                                     start=(ki == 0), stop=(ki == 31))

            def emit_qb_tail(qb, po):
                qsl = slice(512 * qb, 512 * (qb + 1))
                with nc.allow_low_precision(reason="f32r rounding of softmax recip"):
                    nc.vector.reciprocal(rball[64:65, :], po[64:65, :])
                rps = ps1.tile([64, 512], f32, tag="t", name=f"rps{qb}")
                nc.tensor.matmul(rps[:], o64[64:65, :], rball[64:65, :],
                                 start=True, stop=True)
                nc.vector.tensor_copy(rsb[0:64, 512 * (qb % 2):512 * (qb % 2) + 512],
                                      rps[:])
                nc.vector.tensor_tensor(ot[:, qsl], po[0:64, :],
                                        rsb[0:64, 512 * (qb % 2):512 * (qb % 2) + 512],
                                        op=OP.mult)
                nc.sync.dma_start(a2a_in[qb], ot[:, qsl])

            # qb0: just-in-time producers so the PE order matches dataflow.
            # All V batches allocate their psum (psO pool) before po0 so the
            # long-lived po0 accumulator never blocks a V batch.
            emit_k(0)
            emit_q(0)
            emit_st_exp(0, 0)
            emit_k(1)
            emit_k(2)
            emit_st_exp(0, 1)
            emit_k(3)
            emit_st_exp(0, 2)
            emit_k(4)
            emit_st_exp(0, 3)
            emit_k(5)
            emit_vbatch(0)
            emit_st_exp(0, 4)
            emit_vbatch(1)
            emit_st_exp(0, 5)
            emit_k(6)
            emit_vbatch(2)
            emit_st_exp(0, 6)
            emit_k(7)
            emit_vbatch(3)
            emit_st_exp(0, 7)
            po = psO.tile([65, 512], f32, tag="po", name="po0")
            emit_pv(0, 0, po)
            emit_pv(0, 1, po)
            emit_pv(0, 2, po)
            emit_st_exp(0, 8)
            emit_pv(0, 3, po)
            emit_pv(0, 4, po)
            emit_q(1)
            emit_st_exp(0, 9)
            emit_pv(0, 5, po)
            emit_pv(0, 6, po)
            emit_st_exp(0, 10)
            emit_st_exp(1, 0)
            emit_pv(0, 7, po)
            emit_pv(0, 8, po)
            emit_st_exp(1, 1)
            emit_pv(0, 9, po)
            emit_pv(0, 10, po)
            po_prev = po
            for qb in range(1, 8):
                po = psO.tile([65, 512], f32, tag="po", name=f"po{qb}")
                emit_qb_tail(qb - 1, po_prev)
                emit_pv(qb, 0, po)
                emit_pv(qb, 1, po)
                for gi in range(2, NG):
                    emit_st_exp(qb, gi)
                    emit_pv(qb, gi, po)
                    if gi == 5 and qb < 7:
                        emit_q(qb + 1)
                    if gi == 10 and qb < 7:
                        emit_st_exp(qb + 1, 0)
                        emit_st_exp(qb + 1, 1)
                po_prev = po
            emit_qb_tail(7, po)

            # ---------- phase D: all-to-all + proj + residual ----------
            for t in range(4):
                nc.sync.dma_start(xs[t][:], xs_d.ap()[128 * t:128 * (t + 1), :])
                for oi in range(4):
                    nc.sync.dma_start(pw[t][oi][:], pw_d.ap()[t, oi])
            nc.sync.dma_start(pb[:], pb_d.ap())
            if with_collective:
                nc.gpsimd.collective_compute(
                    "AllToAll", mybir.AluOpType.bypass,
                    replica_groups=[list(range(N_CORES))],
                    ins=[a2a_in.opt()], outs=[a2a_out.opt()])
            else:
                nc.sync.dma_start(a2a_out[:], a2a_in[:])
            # keep the PE clock warm through the collective so the proj
            # matmuls don't pay the HAM cold-clock penalty
            warm = ps1.tile([64, 512], f32, tag="t", name="warm")
            for i in range(35):
                nc.tensor.matmul(warm[:], o64[64:65, :], rball[64:65, :],
                                 start=(i == 0), stop=(i == 34))
            og = [T([128, PXS], bf16, name=f"og{ci}") for ci in range(4)]
            gat = a2a_out[:].rearrange("j p e -> (j p) e")
            for ci in range(4):
                nc.sync.dma_start(og[ci][:], gat[128 * ci:128 * (ci + 1), :])
            for oi in range(4):
                pp = (psSa if oi % 2 == 0 else psSb).tile([128, 512], f32, tag="s", name=f"pp{oi}")
                for ci in range(4):
                    nc.tensor.matmul(pp[:], pw[ci][oi][:], og[ci][:],
                                     start=(ci == 0), stop=(ci == 3))
                o_sb = T([128, PXS], f32, name=f"o_sb{oi}")
                nc.vector.scalar_tensor_tensor(o_sb[:], pp[:], pb[:, oi:oi + 1],
                                               xs[oi][:], op0=OP.add, op1=OP.add)
                nc.sync.dma_start(out_d.ap()[128 * oi:128 * (oi + 1), :], o_sb[:])

    nc.compile()
    return nc


def _host_prep(x, norm_w, norm_b, qkv_w, qkv_b, proj_w, proj_b):
    """Build the per-core input maps (all host work is slicing/transposing)."""
    import ml_dtypes
    x2d = np.ascontiguousarray(x.reshape(C, HW).astype(np.float32))
    x2d_bf = x2d.astype(ml_dtypes.bfloat16)
    norm_w = norm_w.astype(np.float32)
    norm_b = norm_b.astype(np.float32)
    qkv_w = qkv_w.astype(np.float32)
    qkv_b = qkv_b.astype(np.float32)
    proj_w = proj_w.astype(np.float32)
    proj_b = proj_b.astype(np.float32)

    # shared constants
    g4 = np.zeros((4, 128, 32), np.float32)
    b4 = np.zeros((4, 32, 128), np.float32)
    for t in range(4):
        gv = 1.0 / (16.0 * 4096.0) if t < 2 else 1.0 / 16.0
        for r in range(128):
            g = (128 * t + r) // 16
            g4[t, r, g] = gv
            b4[t, g, r] = 1.0
    import ml_dtypes
    onesc = np.ones((128, 32), ml_dtypes.bfloat16)
    pw = np.zeros((4, 4, 128, 128), ml_dtypes.bfloat16)
    for ci in range(4):
        for oi in range(4):
            pw[ci, oi] = proj_w[128 * oi:128 * (oi + 1),
                                128 * ci:128 * (ci + 1)].T
    pb = np.zeros((128, 4), np.float32)
    for oi in range(4):
        pb[:, oi] = proj_b[128 * oi:128 * (oi + 1)]

    in_maps = []
    for h in range(N_CORES):
        Wq = qkv_w[HD * h:HD * (h + 1)]
        Wk = qkv_w[C + HD * h:C + HD * (h + 1)]
        Wv = qkv_w[2 * C + HD * h:2 * C + HD * (h + 1)]
        bq = qkv_b[HD * h:HD * (h + 1)]
        bk = qkv_b[C + HD * h:C + HD * (h + 1)]
        bv = qkv_b[2 * C + HD * h:2 * C + HD * (h + 1)]
        scale = HD ** -0.5
        Wq_f = scale * Wq * norm_w[None, :]
        bq_f = scale * (bq + Wq @ norm_b)
        Wk_f = Wk * norm_w[None, :]
        bk_f = bk + Wk @ norm_b
        Wv_f = Wv * norm_w[None, :]
        bv_f = bv + Wv @ norm_b
        wq2 = np.zeros((4, 128, 128), ml_dtypes.bfloat16)
        wk2 = np.zeros((4, 128, 128), ml_dtypes.bfloat16)
        wv_t = np.zeros((4, 128, 64), ml_dtypes.bfloat16)
        for t in range(4):
            cs = slice(128 * t, 128 * (t + 1))
            wq2[t] = np.concatenate([Wq_f[:, cs].T, Wq_f[:, cs].T], axis=1)
            wk2[t] = np.concatenate([Wk_f[:, cs].T, Wk_f[:, cs].T], axis=1)
            wv_t[t] = Wv_f[:, cs].T
        bvr = np.zeros((1, 65), np.float32)
        bvr[0, :64] = bv_f
        bvr[0, 64] = 1.0
        in_maps.append({
            "x_r": x2d_bf,
            "xs": np.ascontiguousarray(x2d[:, PXS * h:PXS * (h + 1)]),
            "wq2": wq2, "wk2": wk2, "wv": wv_t,
            "bq2": np.concatenate([bq_f, bq_f])[:, None].astype(np.float32),
            "bk2": np.concatenate([bk_f, bk_f])[:, None].astype(np.float32),
            "bvr": bvr, "g4": g4, "b4": b4, "onesc": onesc,
            "o64": np.ones((1, 64), np.float32),
            "onesr": np.ones((1, 128), np.float32),
            "pw": pw, "pb": pb,
        })
    return in_maps


def kernel(x, norm_w, norm_b, qkv_w, qkv_b, proj_w, proj_b):
    from concourse.bass_utils import run_bass_kernel_spmd

    if "nc" not in _CACHE:
        _CACHE["nc"] = build(with_collective=True)
    nc = _CACHE["nc"]
    in_maps = _host_prep(np.asarray(x), np.asarray(norm_w), np.asarray(norm_b),
                         np.asarray(qkv_w), np.asarray(qkv_b),
                         np.asarray(proj_w), np.asarray(proj_b))
    res = run_bass_kernel_spmd(nc, in_maps, core_ids=list(range(N_CORES)))
    out = np.concatenate([res.results[h]["out"] for h in range(N_CORES)], axis=1)
    return out.reshape(1, C, 64, 64).astype(np.float32)

